# revision 21
# baseline (speedup 1.0000x reference)
"""Trainium2 Bass kernel for nn_G_HGNN_layer_38448547234609 (v4).

HGNN layer: knn-hypergraph construction (top-11 nearest of 8192 nodes) +
static local-window hyperedges, G = Dv^-1/2 H De^-1 H^T Dv^-1/2 message
passing, linear -> G matmul -> batchnorm(train) -> relu -> residual.

Never materializes G: z = dv2 * (Hfull @ (1/DE * (Hfull^T @ (dv2*y)))).
Sharding: core c owns sample c (1024 nodes = 8 row-tiles of 128).

Distances use a split-fp16 3-pass scheme (x = h1 + h2, ~22 mantissa
bits): host-checked to reproduce the fp32 top-11 exactly. The
transposed pass in P5 uses the same operand pairs in the same order
with lhsT/rhs swapped -> bit-identical PSUM values, so the threshold
element (d == T) lands consistently in both mask orientations.

v4 vs v3 (which measured entirely at the PE's cold 1.2 GHz clock):
 - explicit PE warmup burst (dense independent matmuls) to trip the
   HAM clock gate to 2.4 GHz before real work
 - phases packed to keep the PE dense: local-t folded into P0; the
   first 16 transposed-distance chunks interleaved with P2's u
   matmuls so the u AllReduce window is full of PE work
 - one-tile skew for u matmuls, 16-chunk skew for z^T matmuls (deep
   maskT buffer reuses the SBUF freed by P1's distance rows)
 - u AllReduce payloads in bf16 (counts <= 256 are exact), masks
   spilled to DRAM only for the j-half P2 actually re-reads
"""

import numpy as np
import ml_dtypes

import concourse.bass as bass
import concourse.bacc as bacc
import concourse.mybir as mybir
import concourse.tile as tile
from concourse import bass_utils

AF = mybir.ActivationFunctionType
ALU = mybir.AluOpType
F32 = mybir.dt.float32
F16 = mybir.dt.float16
FP8 = mybir.dt.float8e4
BF16 = mybir.dt.bfloat16

NODE, K, KER, STR = 32, 10, 5, 2
B, C = 8, 64
N = NODE * NODE            # 1024 nodes/sample
BN = B * N                 # 8192
OUT_ = (NODE - KER) // STR + 1
E = OUT_ * OUT_            # 196 local hyperedges/sample
NCORE = 8
NT = 8                     # 128-row tiles per core
JC = 64                    # 128-col j-chunks
LAG = 32                   # z^T matmul skew behind the maskT pipeline
BN_EPS = 1e-5
BIG = 1e30

US = 68                    # u slot stride -> 16B-aligned PSUM writes
UPB = 7                    # u slots per PSUM bank
UW = 65                    # used width per slot (64 ch + count)
UHALF = 4 * 476 + 272      # drained width per half (2176)

_CACHE = {}


def _u_off(slot):
    return (slot // UPB) * 512 + (slot % UPB) * US


def _u_sb_off(slot):
    return (slot // UPB) * 476 + (slot % UPB) * US


def _local_incidence():
    idx = np.arange(N).reshape(NODE, NODE)
    H_local = np.zeros((N, E), np.float32)
    e = 0
    for i in range(0, NODE - KER + 1, STR):
        for j in range(0, NODE - KER + 1, STR):
            H_local[idx[i:i + KER, j:j + KER].ravel(), e] = 1.0
            e += 1
    return H_local


def _build():
    nc = bacc.Bacc(num_devices=NCORE)

    bzp1 = nc.dram_tensor("bzp1", [128, BN], F16, kind="ExternalInput")
    bzp2 = nc.dram_tensor("bzp2", [128, BN], F16, kind="ExternalInput")
    acp1 = nc.dram_tensor("acp1", [128, N], F16, kind="ExternalInput")
    acp2 = nc.dram_tensor("acp2", [128, N], F16, kind="ExternalInput")
    wbp1 = nc.dram_tensor("wbp1", [128, C], F16, kind="ExternalInput")
    wbp2 = nc.dram_tensor("wbp2", [128, C], F16, kind="ExternalInput")
    dv2t = nc.dram_tensor("dv2t", [128, NT], F32, kind="ExternalInput")
    dv2bc = nc.dram_tensor("dv2bc", [C, N], F32, kind="ExternalInput")
    hloc = nc.dram_tensor("hloc", [128, NT * E], BF16, kind="ExternalInput")
    hloct = nc.dram_tensor("hloct", [98, 2 * N], BF16, kind="ExternalInput")
    gamma = nc.dram_tensor("gamma", [C, 1], F32, kind="ExternalInput")
    beta = nc.dram_tensor("beta", [C, 1], F32, kind="ExternalInput")
    identf = nc.dram_tensor("identf", [128, 128], F32, kind="ExternalInput")
    identb = nc.dram_tensor("identb", [C, C], BF16, kind="ExternalInput")
    xres = nc.dram_tensor("xres", [128, NT * C], F32, kind="ExternalInput")
    out = nc.dram_tensor("out", [N, C], F32, kind="ExternalOutput")

    with tile.TileContext(nc) as tc:
        with (
            tc.tile_pool(name="const", bufs=1) as cp,
            tc.tile_pool(name="small", bufs=4) as sp,
            tc.tile_pool(name="persist", bufs=1) as pp,
            tc.tile_pool(name="dram", bufs=1, space="DRAM") as dr,
        ):
            # ---- PE warmup: dense independent matmuls (no DMA deps) ----
            wu = pp.tile([128, 512], BF16, tag="warm")
            nc.vector.memset(wu[:], 1.0)
            with tc.tile_pool(name="pw", bufs=2, space="PSUM") as pwp:
                for w in range(16):
                    w_ps = pwp.tile([128, 512], F32, tag="wps")
                    nc.tensor.matmul(w_ps[:], lhsT=wu[:, 0:128], rhs=wu[:],
                                     start=True, stop=True)

            # ---- const loads ----
            b1_sb = cp.tile([128, BN], F16, tag="bzp1")
            b2_sb = cp.tile([128, BN], F16, tag="bzp2")
            # quarter loads: P1 tile 0 can start on the first j-quarter
            for q in range(4):
                qs = slice(q * (BN // 4), (q + 1) * (BN // 4))
                nc.sync.dma_start(b1_sb[:, qs], bzp1[:, qs])
                nc.sync.dma_start(b2_sb[:, qs], bzp2[:, qs])
            a1_sb = cp.tile([128, N], F16, tag="acp1")
            nc.sync.dma_start(a1_sb[:], acp1[:])
            a2_sb = cp.tile([128, N], F16, tag="acp2")
            nc.sync.dma_start(a2_sb[:], acp2[:])
            wb1_sb = cp.tile([128, C], F16, tag="wbp1")
            nc.sync.dma_start(wb1_sb[:], wbp1[:])
            wb2_sb = cp.tile([128, C], F16, tag="wbp2")
            nc.sync.dma_start(wb2_sb[:], wbp2[:])
            dv2_sb = cp.tile([128, NT], F32, tag="dv2")
            nc.sync.dma_start(dv2_sb[:], dv2t[:])
            dv2bc_sb = cp.tile([C, N], F32, tag="dv2bc")
            nc.sync.dma_start(dv2bc_sb[:], dv2bc[:])
            hloc_sb = cp.tile([128, NT * E], BF16, tag="hloc")
            nc.sync.dma_start(hloc_sb[:], hloc[:])
            hloct_sb = cp.tile([98, 2 * N], BF16, tag="hloct")
            nc.sync.dma_start(hloct_sb[:], hloct[:])
            gam_sb = cp.tile([C, 1], F32, tag="gamma")
            nc.sync.dma_start(gam_sb[:], gamma[:])
            bet_sb = cp.tile([C, 1], F32, tag="beta")
            nc.sync.dma_start(bet_sb[:], beta[:])
            xr_sb = cp.tile([128, NT * C], F32, tag="xres")
            nc.sync.dma_start(xr_sb[:], xres[:])
            idf_sb = cp.tile([128, 128], F32, tag="identf")
            nc.sync.dma_start(idf_sb[:], identf[:])
            idb_sb = cp.tile([C, C], BF16, tag="identb")
            nc.sync.dma_start(idb_sb[:], identb[:])

            m_aug = pp.tile([128, NT * 65], BF16, tag="maug")
            t_tiles = pp.tile([128, NT], F32, tag="ttiles")
            tb_sb = pp.tile([128, N], F32, tag="tbcast")
            trow_sb = pp.tile([1, N], F32, tag="trow")
            ones_sb = pp.tile([1, 128], F32, tag="ones1")
            nc.vector.memset(ones_sb[:], 1.0)
            u_sb = pp.tile([128, UHALF], BF16, tag="usb")
            ur_sb = pp.tile([128, UHALF], BF16, tag="ursb")
            v_sb = pp.tile([128, JC * C], BF16, tag="vsb")
            vloc_sb = pp.tile([98, 2 * C], BF16, tag="vloc")
            zs_sb = pp.tile([C, N], F32, tag="zssb")
            rl_sb = pp.tile([C, N], BF16, tag="rlsb")

            mask_dram = [dr.tile([128, BN // 2], BF16, tag=f"mask{i}",
                                 name=f"mask_dram{i}") for i in range(NT)]
            cc_inA = dr.tile([128, UHALF], BF16, tag="ccinA")
            cc_outA = dr.tile([128, UHALF], BF16, tag="ccoutA", addr_space="Shared")
            cc_inB = dr.tile([128, UHALF], BF16, tag="ccinB")
            cc_outB = dr.tile([128, UHALF], BF16, tag="ccoutB", addr_space="Shared")

            # ---- P0: y = x W^T + b (split fp16); m = dv2*y; local t ----
            with (
                tc.tile_pool(name="py", bufs=2, space="PSUM") as pyp,
                tc.tile_pool(name="ptl", bufs=1, space="PSUM") as ptlp,
            ):
                tl_ps = [ptlp.tile([98, C], F32, tag=f"tl{ec}", name=f"tl_ps{ec}")
                         for ec in range(2)]
                for it in range(NT):
                    y_ps = pyp.tile([128, C], F32, tag="y")
                    nc.tensor.matmul(y_ps[:], lhsT=a1_sb[:, it * 128:(it + 1) * 128],
                                     rhs=wb1_sb[:], start=True, stop=False)
                    nc.tensor.matmul(y_ps[:], lhsT=a2_sb[:, it * 128:(it + 1) * 128],
                                     rhs=wb2_sb[:], start=False, stop=True)
                    nc.scalar.activation(m_aug[:, it * 65:it * 65 + C], y_ps[:],
                                         AF.Copy, bias=0.0, scale=dv2_sb[:, it:it + 1])
                    nc.vector.memset(m_aug[:, it * 65 + C:it * 65 + 65], 1.0)
                    for ec in range(2):
                        nc.tensor.matmul(tl_ps[ec][:],
                                         lhsT=hloc_sb[:, it * E + ec * 98:it * E + ec * 98 + 98],
                                         rhs=m_aug[:, it * 65:it * 65 + C],
                                         start=(it == 0), stop=(it == NT - 1))
                for ec in range(2):
                    nc.scalar.activation(vloc_sb[:, ec * C:(ec + 1) * C], tl_ps[ec][:],
                                         AF.Copy, bias=0.0, scale=1.0 / 25.0)

            uorder = [b * UPB + s for s in range(UPB) for b in range(5) if b * UPB + s < 32]

            def u_matmuls(it, mk, base):
                for jc in uorder:
                    o = _u_off(jc)
                    nc.tensor.matmul(u_ps[:, o:o + UW],
                                     lhsT=mk[:, (jc + base) * 128:(jc + base + 1) * 128],
                                     rhs=m_aug[:, it * 65:(it + 1) * 65],
                                     start=False, stop=(it == NT - 1),
                                     skip_group_check=True)

            def u_drain(cc_in, cc_out):
                for b in range(5):
                    w = 476 if b < 4 else 272
                    nc.scalar.copy(u_sb[:, b * 476:b * 476 + w],
                                   u_ps[:, b * 512:b * 512 + w])
                nc.sync.dma_start(cc_in[:], u_sb[:])
                nc.gpsimd.collective_compute(
                    "AllReduce", ALU.add, replica_groups=[list(range(NCORE))],
                    ins=[cc_in.opt()], outs=[cc_out.opt()])

            def v_extract(ur, j0):
                for jc in range(32):
                    o = _u_sb_off(jc)
                    rec = sp.tile([128, 1], F32, tag="rec")
                    nc.vector.reciprocal(rec[:], ur[:, o + C:o + UW])
                    nc.vector.tensor_scalar(v_sb[:, (j0 + jc) * C:(j0 + jc + 1) * C],
                                            ur[:, o:o + C],
                                            rec[:, 0:1], None, ALU.mult)

            with tc.tile_pool(name="pu", bufs=1, space="PSUM") as pup:
                u_ps = pup.tile([128, 5 * 512], F32, tag="u")
                nc.vector.memset(u_ps[:], 0.0)

                # ---- P1: distances, threshold, mask, u chunks 0..31 ----
                with (
                    tc.tile_pool(name="pd", bufs=3, space="PSUM") as pdp,
                    tc.tile_pool(name="dwork", bufs=2) as dp,
                    tc.tile_pool(name="mworkL", bufs=2) as mpl,
                    tc.tile_pool(name="mworkR", bufs=1) as mpr,
                ):
                    mks = []
                    for it in range(NT):
                        a1t = a1_sb[:, it * 128:(it + 1) * 128]
                        a2t = a2_sb[:, it * 128:(it + 1) * 128]
                        d = dp.tile([128, BN], F32, tag="d")
                        cand = sp.tile([128, 128], F32, tag="cand")
                        for nck in range(16):
                            d_ps = pdp.tile([128, 512], F32, tag="dch")
                            rb1 = b1_sb[:, nck * 512:(nck + 1) * 512]
                            rb2 = b2_sb[:, nck * 512:(nck + 1) * 512]
                            nc.tensor.matmul(d_ps[:], lhsT=a1t, rhs=rb1, start=True, stop=False)
                            nc.tensor.matmul(d_ps[:], lhsT=a2t, rhs=rb2, start=False, stop=True)
                            nc.scalar.copy(d[:, nck * 512:(nck + 1) * 512], d_ps[:])
                            nc.vector.max(cand[:, nck * 8:(nck + 1) * 8],
                                          d[:, nck * 512:(nck + 1) * 512])
                        c8a = sp.tile([128, 8], F32, tag="v8")
                        nc.vector.max(c8a[:], cand[:])
                        nc.vector.match_replace(cand[:], c8a[:], cand[:], -BIG)
                        c8b = sp.tile([128, 8], F32, tag="v8")
                        nc.vector.max(c8b[:], cand[:])
                        nc.scalar.copy(t_tiles[:, it:it + 1], c8b[:, 2:3])
                        mkl = mpl.tile([128, BN // 2], BF16, tag="mkl")
                        nc.vector.tensor_scalar(mkl[:], d[:, :BN // 2],
                                                c8b[:, 2:3], None, ALU.is_ge)
                        mkr1 = mpr.tile([128, BN // 2], BF16, tag="mkr")
                        nc.vector.tensor_scalar(mkr1[:], d[:, BN // 2:],
                                                c8b[:, 2:3], None, ALU.is_ge)
                        # only the right j-half is re-read (by P2)
                        nc.sync.dma_start(mask_dram[it][:], mkr1[:])
                        mks.append(mkl)
                        if it > 0:
                            u_matmuls(it - 1, mks[it - 1], 0)
                    u_matmuls(NT - 1, mks[NT - 1], 0)
                u_drain(cc_inA, cc_outA)

                # ---- Tbcast: T row broadcast to 128 partitions (exact) ----
                with tc.tile_pool(name="ptb", bufs=1, space="PSUM") as ptbp:
                    tt_ps = ptbp.tile([NT, 128], F32, tag="ttp")
                    nc.tensor.transpose(tt_ps[:], t_tiles[:, 0:NT], idf_sb[:])
                    tt_sb = sp.tile([NT, 128], F32, tag="tts")
                    nc.scalar.copy(tt_sb[:], tt_ps[:])
                    for it in range(NT):
                        nc.sync.dma_start(trow_sb[0:1, it * 128:(it + 1) * 128],
                                          tt_sb[it:it + 1, :])
                    for h in range(2):
                        tb_ps = ptbp.tile([128, 512], F32, tag="tbp")
                        nc.tensor.matmul(tb_ps[:], lhsT=ones_sb[:],
                                         rhs=trow_sb[:, h * 512:(h + 1) * 512],
                                         start=True, stop=True)
                        nc.scalar.copy(tb_sb[:, h * 512:(h + 1) * 512], tb_ps[:])

                # P5 transposed-distance machinery (starts during P2).
                # The pool opens after P1 so it reuses the distance-row SBUF.
                mtw_cm = tc.tile_pool(name="mtw", bufs=LAG + 2)
                mtp = mtw_cm.__enter__()
                mts = []

                def dt_chunk(jc, pool, bufs_note=None):
                    l1 = b1_sb[:, jc * 128:(jc + 1) * 128]
                    l2 = b2_sb[:, jc * 128:(jc + 1) * 128]
                    dt_ps = pool.tile([128, N], F32, tag="dt")
                    for h in range(2):
                        ra1 = a1_sb[:, h * 512:(h + 1) * 512]
                        ra2 = a2_sb[:, h * 512:(h + 1) * 512]
                        o = dt_ps[:, h * 512:(h + 1) * 512]
                        nc.tensor.matmul(o, lhsT=l1, rhs=ra1, start=True, stop=False,
                                         skip_group_check=True)
                        nc.tensor.matmul(o, lhsT=l2, rhs=ra2, start=False, stop=True,
                                         skip_group_check=True)
                    mt = mtp.tile([128, N], FP8, tag="mt")
                    nc.vector.scalar_tensor_tensor(mt[:], dt_ps[:], 0.0, tb_sb[:],
                                                   ALU.add, ALU.is_ge)
                    mts.append(mt)

                # ---- P2 interleaved with dT chunks 0..15 ----
                nc.vector.memset(u_ps[:], 0.0)
                with (
                    tc.tile_pool(name="pdtA", bufs=1, space="PSUM") as pdtA,
                    tc.tile_pool(name="mhalf", bufs=2) as mhp,
                ):
                    mkr = []
                    for it in range(2):
                        mk = mhp.tile([128, BN // 2], BF16, tag="mkh")
                        nc.sync.dma_start(mk[:], mask_dram[it][:])
                        mkr.append(mk)
                    for it in range(NT):
                        dt_chunk(it, pdtA)
                        u_matmuls(it, mkr[it], 0)
                        if it + 2 < NT:
                            mk = mhp.tile([128, BN // 2], BF16, tag="mkh")
                            nc.sync.dma_start(mk[:], mask_dram[it + 2][:])
                            mkr.append(mk)
                u_drain(cc_inB, cc_outB)
            # pu closed; PSUM free for the main P5 loop

            # ---- P5 main loop: dT chunks 16..63, z^T lagged by LAG ----
            with (
                tc.tile_pool(name="pdtB", bufs=2, space="PSUM") as pdtB,
                tc.tile_pool(name="pz", bufs=1, space="PSUM") as pzp,
            ):
                zt_ps = pzp.tile([C, N], F32, tag="zt")
                # local hyperedge part opens the accumulation group
                for ec in range(2):
                    for h in range(2):
                        nc.tensor.matmul(zt_ps[:, h * 512:(h + 1) * 512],
                                         lhsT=vloc_sb[:, ec * C:(ec + 1) * C],
                                         rhs=hloct_sb[:, ec * N + h * 512:ec * N + (h + 1) * 512],
                                         start=(ec == 0), stop=False,
                                         skip_group_check=True)

                def zt_matmuls(prev):
                    mt_ = mts[prev]
                    for h in range(2):
                        nc.tensor.matmul(zt_ps[:, h * 512:(h + 1) * 512],
                                         lhsT=v_sb[:, prev * C:(prev + 1) * C],
                                         rhs=mt_[:, h * 512:(h + 1) * 512],
                                         start=False, stop=False,
                                         skip_group_check=True)

                for jc in range(NT, JC):
                    if jc == 24:
                        nc.sync.dma_start(ur_sb[:], cc_outA[:])
                        v_extract(ur_sb, 0)
                    dt_chunk(jc, pdtB)
                    if jc >= LAG:
                        zt_matmuls(jc - LAG)
                nc.sync.dma_start(ur_sb[:], cc_outB[:])
                v_extract(ur_sb, 32)
                for prev in range(JC - LAG, JC - 1):
                    zt_matmuls(prev)
                mt_ = mts[JC - 1]
                for h in range(2):
                    nc.tensor.matmul(zt_ps[:, h * 512:(h + 1) * 512],
                                     lhsT=v_sb[:, (JC - 1) * C:JC * C],
                                     rhs=mt_[:, h * 512:(h + 1) * 512],
                                     start=False, stop=True,
                                     skip_group_check=True)
                # ---- BN stats straight from PSUM: zs = dv2 * z^T ----
                nc.vector.tensor_tensor(zs_sb[:], zt_ps[:], dv2bc_sb[:], ALU.mult)
            mtw_cm.__exit__(None, None, None)
            s1 = sp.tile([C, 1], F32, tag="s1")
            nc.vector.tensor_reduce(s1[:], zs_sb[:], mybir.AxisListType.X, ALU.add)
            s2 = sp.tile([C, 1], F32, tag="s2")
            # rl_sb is scratch here (the relu overwrites it later)
            nc.vector.scalar_tensor_tensor(rl_sb[:], zs_sb[:], 0.0, zs_sb[:],
                                           ALU.add, ALU.mult, accum_out=s2[:])
            st_sb = sp.tile([C, 2], F32, tag="stsb")
            nc.vector.tensor_copy(st_sb[:, 0:1], s1[:])
            nc.vector.tensor_copy(st_sb[:, 1:2], s2[:])
            st_in = dr.tile([C, 2], F32, tag="stin")
            st_out = dr.tile([C, 2], F32, tag="stout", addr_space="Shared")
            nc.sync.dma_start(st_in[:], st_sb[:])
            nc.gpsimd.collective_compute(
                "AllReduce", ALU.add, replica_groups=[list(range(NCORE))],
                ins=[st_in.opt()], outs=[st_out.opt()])
            stg = sp.tile([C, 2], F32, tag="stg")
            nc.sync.dma_start(stg[:], st_out[:])

            mu = sp.tile([C, 1], F32, tag="mu")
            nc.vector.tensor_scalar(mu[:], stg[:, 0:1], 1.0 / BN, None, ALU.mult)
            ex2 = sp.tile([C, 1], F32, tag="ex2")
            nc.vector.tensor_scalar(ex2[:], stg[:, 1:2], 1.0 / BN, None, ALU.mult)
            musq = sp.tile([C, 1], F32, tag="musq")
            nc.vector.tensor_tensor(musq[:], mu[:], mu[:], ALU.mult)
            var = sp.tile([C, 1], F32, tag="var")
            nc.vector.tensor_tensor(var[:], ex2[:], musq[:], ALU.subtract)
            eps_t = sp.tile([C, 1], F32, tag="eps")
            nc.vector.memset(eps_t[:], BN_EPS)
            sd = sp.tile([C, 1], F32, tag="sd")
            nc.scalar.activation(sd[:], var[:], AF.Sqrt, bias=eps_t[:, 0:1], scale=1.0)
            inv = sp.tile([C, 1], F32, tag="inv")
            nc.vector.reciprocal(inv[:], sd[:])
            srow = sp.tile([C, 1], F32, tag="srow")
            nc.vector.tensor_tensor(srow[:], gam_sb[:], inv[:], ALU.mult)
            msr = sp.tile([C, 1], F32, tag="msr")
            nc.vector.tensor_tensor(msr[:], mu[:], srow[:], ALU.mult)
            trow = sp.tile([C, 1], F32, tag="trw")
            nc.vector.tensor_tensor(trow[:], bet_sb[:], msr[:], ALU.subtract)

            # ---- P7: relu(zs*s + t) in z^T, transpose, residual, out ----
            nc.scalar.activation(rl_sb[:], zs_sb[:], AF.Relu,
                                 bias=trow[:, 0:1], scale=srow[:, 0:1])
            with tc.tile_pool(name="pot", bufs=2, space="PSUM") as potp:
                for it in range(NT):
                    o_ps = potp.tile([128, C], BF16, tag="ot")
                    nc.tensor.transpose(o_ps[:], rl_sb[:, it * 128:(it + 1) * 128], idb_sb[:])
                    ot = sp.tile([128, C], F32, tag="ots")
                    nc.vector.tensor_tensor(ot[:], o_ps[:],
                                            xr_sb[:, it * C:(it + 1) * C], ALU.add)
                    nc.sync.dma_start(out[it * 128:(it + 1) * 128, :], ot[:])

    nc.compile()
    return nc


def _host_inputs(x, W_conv, b_conv, gamma, beta):
    xm = np.ascontiguousarray(x.reshape(BN, C).astype(np.float32))
    xT = np.ascontiguousarray(xm.T)
    sq = (xm * xm).sum(1).astype(np.float32)

    f16 = np.float16
    x2T = 2.0 * xT
    b1 = x2T.astype(f16)
    b2 = (x2T - b1.astype(np.float32)).astype(f16)
    s1 = (-sq).astype(f16)
    s2 = (-sq - s1.astype(np.float32)).astype(f16)
    zbn = np.zeros((62, BN), f16)
    # pass1: h1.b1 + 1*s1 + 1*s2 (rows 66-127 zero); pass2: h1.b2 + h2.b1
    bzp1 = np.concatenate([b1, s1[None, :], s2[None, :], zbn], 0)
    bzp2 = np.concatenate([b2, b1], 0)

    h1T = xT.astype(f16)
    h2T = (xT - h1T.astype(np.float32)).astype(f16)

    wT16 = W_conv.T.astype(np.float32).astype(f16)
    wbp1 = np.concatenate([wT16, b_conv[None, :].astype(f16),
                           np.zeros((63, C), f16)], 0)
    wbp2 = np.concatenate([np.zeros((64, C), f16), wT16], 0)

    H_local = _local_incidence()
    cover = H_local.sum(1)
    dv2 = ((K + 1 + cover) ** -0.5).astype(np.float32)
    dv2t = dv2.reshape(NT, 128).T.copy()
    dv2bc = np.broadcast_to(dv2[None, :], (C, N)).copy()

    hloc = np.zeros((128, NT * E), np.float32)
    for it in range(NT):
        hloc[:, it * E:(it + 1) * E] = H_local[it * 128:(it + 1) * 128, :]
    hloct = np.zeros((98, 2 * N), np.float32)
    for ec in range(2):
        hloct[:, ec * N:(ec + 1) * N] = H_local[:, ec * 98:(ec + 1) * 98].T

    bf = ml_dtypes.bfloat16
    common = {
        "bzp1": bzp1,
        "bzp2": bzp2,
        "wbp1": wbp1,
        "wbp2": wbp2,
        "dv2t": dv2t,
        "dv2bc": dv2bc,
        "hloc": hloc.astype(bf),
        "hloct": hloct.astype(bf),
        "gamma": np.ascontiguousarray(gamma.astype(np.float32)[:, None]),
        "beta": np.ascontiguousarray(beta.astype(np.float32)[:, None]),
        "identf": np.eye(128, dtype=np.float32),
        "identb": np.eye(C, dtype=np.float32).astype(bf),
    }
    in_maps = []
    zn1 = np.zeros((62, N), f16)
    for c in range(NCORE):
        cs = slice(c * N, (c + 1) * N)
        acp1 = np.concatenate([h1T[:, cs], np.ones((2, N), f16), zn1], 0)
        acp2 = np.concatenate([h1T[:, cs], h2T[:, cs]], 0)
        xr = np.zeros((128, NT * C), np.float32)
        for it in range(NT):
            xr[:, it * C:(it + 1) * C] = xm[c * N + it * 128:c * N + (it + 1) * 128, :]
        m = dict(common)
        m["acp1"] = np.ascontiguousarray(acp1)
        m["acp2"] = np.ascontiguousarray(acp2)
        m["xres"] = xr
        in_maps.append(m)
    return in_maps


def _get_nc():
    if "nc" not in _CACHE:
        _CACHE["nc"] = _build()
    return _CACHE["nc"]


def run_spmd(inputs, **kw):
    nc = _get_nc()
    in_maps = _host_inputs(inputs["x"], inputs["W_conv"], inputs["b_conv"],
                           inputs["gamma"], inputs["beta"])
    return bass_utils.run_bass_kernel_spmd(nc, in_maps, core_ids=list(range(NCORE)), **kw)


def kernel(**inputs):
    res = run_spmd(inputs)
    out = np.stack([res.results[c]["out"] for c in range(NCORE)], 0)
    return out.reshape(B, N, C).astype(np.float32)
